# revision 45
# baseline (speedup 1.0000x reference)
"""GCN 2-layer + link decode on 8 TRN2 NeuronCores (full inputs in/out).

Design (dest-sharded, scatter-free, bf16):
- Aggregation commutes with the weight matmul: h = relu(segsum(w1*x[src]) @ W1);
  uv per node = segsum2(w2*h[src]) @ (W2 @ [WlinA.T|WlinB.T]) (4 cols).
- Each core owns 12544 destination slots. Edge streams sorted by
  (src int16-range, dest chunk, dest); bulk-gathered via dma_gather (int16
  local indices per 32768-row range, bf16 256B rows); routed+weighted into
  per-chunk PSUM by a selection-matrix matmul (swapped orientation:
  lhsT=staged rows, rhs=sel -> psum holds A^T[feat, slot], no transposes).
  sel[e,m] = (iota[e,m] == rel[e]) * w[e] built in ONE DVE tensor_scalar op
  (two scalar operands -> 4x_2p fast mode, bf16).
- Inter-layer exchange: each core writes its h/uv chunk slices directly into
  a Shared DRAM table via indirect-offset DMA (per-core row offsets are
  input data), then a tiny flag AllGather into scratch rows acts as the
  cross-core barrier (no 25-50MB AllGathers).
- Decode: pairs sorted by pos0, sharded by index; u and v streams routed
  into one [2, 128] psum per pair chunk (lhsT = uv cols 0:2 / 2:4);
  host unshuffles the [2, 25088] output.
"""
import numpy as np

P = 128
N = 100_000
NSHARD = 12_500
SLOTS = 12_544
CHUNKS = SLOTS // P          # 98
TABROWS = 8 * SLOTS          # 100352
SCR = 16                     # scratch rows for barrier AllGather
RBOUND = [32768, 65536, 98304]
RLO = [0, 32768, 65536, 98304]
NCORES = 8
CALL_CELLS = 8               # chunks per gather-call window


def _range_of(a):
    return np.searchsorted(RBOUND, a, side="right")


def _wrap_idx(a):
    """[NCORES, T] int16 -> [NCORES, 128, T//16] (16-wrap, 8x replicate)."""
    ncr, t = a.shape
    out = a.reshape(ncr, t // 16, 16).transpose(0, 2, 1)
    return np.ascontiguousarray(np.tile(out, (1, 8, 1)))


def _prep_stream(tab_row, slot, w, nchunks, call_cells):
    """Generic SPMD-uniform stream builder.

    tab_row: [E] global table row per entry; slot: [E] local out slot
    (0..nchunks*128); w: [E] weight; entries already per-core-filtered lists:
    tab_row etc are lists of arrays, one per core.
    Returns static schedule + per-core idx16 / rel / w arrays.
    """
    ncr = len(tab_row)
    # cell = (chunk, range); count per core
    counts = np.zeros((ncr, nchunks, 4), np.int64)
    for c in range(ncr):
        ch = slot[c] // P
        rr = _range_of(tab_row[c])
        np.add.at(counts, (c, ch, rr), 1)
    estar = counts.max(axis=0)                       # [nchunks, 4]

    # layout per range: calls of CALL windows, each padded to 128 multiple
    layout = []
    for r in range(4):
        if estar[:, r].sum() == 0:
            layout.append(dict(calls=[], T=0))
            continue
        calls = []
        base = 0
        for k0 in range(0, nchunks, call_cells):
            k1 = min(k0 + call_cells, nchunks)
            cells = estar[k0:k1, r]
            offs = np.concatenate([[0], np.cumsum(cells)]).astype(np.int64)
            n = int(offs[-1])
            n_pad = max(P, ((n + P - 1) // P) * P)
            calls.append(dict(k0=k0, k1=k1, offs=offs, n=n, n_pad=n_pad,
                              base=base))
            base += n_pad
        layout.append(dict(calls=calls, T=base))

    # static schedule: per chunk, matmul descriptors (r, call, blk, sel_col)
    sched = [[] for _ in range(nchunks)]
    selmap = {}
    n_sel = 0
    for r in range(4):
        for ci, call in enumerate(layout[r]["calls"]):
            nblk = call["n_pad"] // P
            offs, k0 = call["offs"], call["k0"]
            for b in range(nblk):
                e0, e1 = b * P, b * P + P
                ks = [k for k in range(call["k0"], call["k1"])
                      if offs[k - k0] < e1 and offs[k - k0 + 1] > e0]
                if not ks:
                    ks = [call["k0"]]
                for k in ks:
                    sched[k].append(dict(r=r, call=ci, blk=b, sel=n_sel))
                    selmap[(r, ci, b, k)] = n_sel
                    n_sel += 1

    idx16 = [np.zeros((ncr, layout[r]["T"]), np.int16) for r in range(4)]
    rel = np.zeros((ncr, P, n_sel), np.float32)
    wgt = np.zeros((ncr, P, n_sel), np.float32)

    for c in range(ncr):
        tr, sl, ww = tab_row[c], slot[c], w[c]
        rr = _range_of(tr)
        ch = sl // P
        # order entries by (range, chunk, slot)
        o = np.lexsort((sl, ch, rr))
        tr, sl, ww, rr, ch = tr[o], sl[o], ww[o], rr[o], ch[o]
        for r in range(4):
            m = rr == r
            if not m.any():
                continue
            trm, slm, wwm, chm = tr[m], sl[m], ww[m], ch[m]
            # position: call base + cell offset + within-cell rank
            cell_cnt = np.zeros(nchunks, np.int64)
            np.add.at(cell_cnt, chm, 1)
            cstart = np.concatenate([[0], np.cumsum(cell_cnt)])
            within = np.arange(len(slm)) - cstart[chm]
            call_id = chm // call_cells
            calls = layout[r]["calls"]
            cbase = np.array([cl["base"] for cl in calls], np.int64)
            # offset of chunk's cell within its call
            cell_off = np.zeros(nchunks, np.int64)
            for ci, cl in enumerate(calls):
                for k in range(cl["k0"], cl["k1"]):
                    cell_off[k] = cl["offs"][k - cl["k0"]]
            pos = cbase[call_id] + cell_off[chm] + within
            idx16[r][c, pos] = (trm - RLO[r]).astype(np.int16)
            # sel column per entry
            relpos = pos - cbase[call_id]
            blk = relpos // P
            pp = relpos % P
            cols = np.array([selmap[(r, int(ci_), int(b_), int(k_))]
                             for ci_, b_, k_ in zip(call_id, blk, chm)],
                            np.int64)
            rel[c, pp, cols] = (slm % P).astype(np.float32)
            wgt[c, pp, cols] = wwm
    return dict(layout=layout, sched=sched, n_sel=n_sel, idx16=idx16,
                rel=rel, wgt=wgt)


def _run_sim(nc, in_maps):
    """CoreSim timeline simulation: correctness + exec-time estimate."""
    import os
    from concourse.bass_interp import MultiCoreSim
    nc.insert_bir_kernel_barrier_sem_inc()
    sim = MultiCoreSim(nc, NCORES, aliases={},
                       require_finite=False, require_nnan=False,
                       trace=bool(os.environ.get("SIM_TRACE")),
                       num_workers=1)
    for c in range(NCORES):
        for name, arr in in_maps[c].items():
            sim.cores[c].tensor(name)[:] = arr
        if nc.partition_id_tensor is not None:
            sim.cores[c].tensor(nc.partition_id_tensor.name)[:] = np.array(
                [[c]], np.uint32)
    sim.simulate()
    globals()["LAST_EXEC_NS"] = int(sim.global_time)
    globals()["SIM_CORE_NS"] = [int(sim.cores[c].time) for c in range(NCORES)]
    if os.environ.get("SIM_DUMP"):
        np.save("/tmp/sim_h_slice.npy", np.stack(
            [np.asarray(sim.cores[c].tensor("h_slice"), np.float32)
             for c in range(NCORES)]))
        np.save("/tmp/sim_uv_red.npy",
                np.asarray(sim.cores[0].tensor("uv_red"), np.float32))
        np.save("/tmp/sim_uv_tab.npy",
                np.asarray(sim.cores[0].tensor("uv_tab"), np.float32))
        np.save("/tmp/sim_out.npy", np.stack(
            [np.asarray(sim.cores[c].tensor("out_dec"))
             for c in range(NCORES)]))
    return [{"out_dec": np.asarray(sim.cores[c].tensor("out_dec"))}
            for c in range(NCORES)]


def kernel(x, edge_index1, edge_index2, edge_weight1, edge_weight2,
           pos_edge_index, W1, W2, Wlin):
    import ml_dtypes
    import concourse.bass as bass
    from concourse import bacc, tile, mybir
    from concourse.bass_utils import run_bass_kernel_spmd
    from concourse.library_config import mlp

    f32, i16, i32 = mybir.dt.float32, mybir.dt.int16, mybir.dt.int32
    bf16 = mybir.dt.bfloat16
    bf = ml_dtypes.bfloat16
    x = np.asarray(x, np.float32)
    W1 = np.asarray(W1, np.float32)
    W2 = np.asarray(W2, np.float32)
    Wlin = np.asarray(Wlin, np.float32)
    e1 = np.asarray(edge_index1).astype(np.int64)
    e2 = np.asarray(edge_index2).astype(np.int64)
    w1 = np.asarray(edge_weight1, np.float32)
    w2 = np.asarray(edge_weight2, np.float32)
    pe = np.asarray(pos_edge_index).astype(np.int64)

    # ---------- host index preprocessing ----------
    x_tab = np.zeros((TABROWS, P), bf)
    x_tab[:N] = x.astype(bf)
    n2row = (np.arange(N) // NSHARD) * SLOTS + (np.arange(N) % NSHARD)

    def shard_by_dest(src_rows, dst, w):
        owner = dst // NSHARD
        ld = dst - owner * NSHARD
        return ([src_rows[owner == c] for c in range(NCORES)],
                [ld[owner == c] for c in range(NCORES)],
                [w[owner == c] for c in range(NCORES)])

    l1 = _prep_stream(*shard_by_dest(e1[0], e1[1], w1), CHUNKS, CALL_CELLS)

    # L2 sharded by SOURCE owner: each core gathers only from its own
    # h_slice; dest slots are global (784 chunks over the n2row space).
    GCHUNKS = TABROWS // P                               # 784
    owner2 = e2[0] // NSHARD
    l2 = _prep_stream(
        [e2[0][owner2 == c] - c * NSHARD for c in range(NCORES)],
        [n2row[e2[1]][owner2 == c] for c in range(NCORES)],
        [w2[owner2 == c] for c in range(NCORES)],
        GCHUNKS, CALL_CELLS)

    # uv tables live in a transposed row order: n2row j -> (j%128)*784+j//128
    def sig(j):
        return (j % P) * GCHUNKS + j // P

    # decode: shard pairs by original index; per core sort by pos0-row and
    # use the local sorted position as the output slot (host unshuffles).
    t0 = sig(n2row[pe[0]])
    t1 = sig(n2row[pe[1]])
    npairs = pe.shape[1]
    pershard = (npairs + NCORES - 1) // NCORES            # 25000
    per_core = ((pershard + P - 1) // P) * P              # 25088
    dchunks = per_core // P
    pair_slot = np.empty(npairs, np.int64)
    u_rows, u_slots, u_w = [], [], []
    v_rows, v_slots, v_w = [], [], []
    for c in range(NCORES):
        p0, p1 = c * pershard, min((c + 1) * pershard, npairs)
        loc = np.argsort(t0[p0:p1], kind="stable")
        sl = np.empty(p1 - p0, np.int64)
        sl[loc] = np.arange(p1 - p0)
        pair_slot[p0:p1] = c * per_core + sl
        ones = np.ones(p1 - p0, np.float32)
        u_rows.append(t0[p0:p1]); u_slots.append(sl); u_w.append(ones)
        v_rows.append(t1[p0:p1]); v_slots.append(sl); v_w.append(ones)
    du = _prep_stream(u_rows, u_slots, u_w, dchunks, 48)
    dv = _prep_stream(v_rows, v_slots, v_w, dchunks, 48)

    idx_arr = {}
    for key, pr in (("l1", l1), ("l2", l2), ("u", du), ("v", dv)):
        for r in range(4):
            if pr["layout"][r]["T"] == 0:
                continue
            idx_arr[(key, r)] = _wrap_idx(pr["idx16"][r])

    # ---------- device program ----------
    nc = bacc.Bacc("TRN2", target_bir_lowering=False, debug=False,
                   num_devices=NCORES, num_swdge_queues=4)

    def din(name, shape, dt=f32):
        return nc.dram_tensor(name, list(shape), dt, kind="ExternalInput").ap()

    xt = din("x_tab", (TABROWS, P), bf16)
    w1t = din("W1r", (P, P), bf16)
    wut = din("Wu", (P, 4), bf16)
    iota_in = din("iota", (P, P), bf16)
    idx_in = {k: din(f"idx_{k[0]}_{k[1]}", v.shape[1:], i16)
              for k, v in idx_arr.items()}
    relw_in = {key: (din(f"rel_{key}", (P, pr["n_sel"])),
                     din(f"w_{key}", (P, pr["n_sel"])))
               for key, pr in (("l1", l1), ("l2", l2), ("u", du), ("v", dv))}

    out_d = nc.dram_tensor("out_dec", [2, per_core], f32,
                           kind="ExternalOutput").ap()
    h_slice = nc.dram_tensor("h_slice", [SLOTS, P], bf16)
    uv_part = nc.dram_tensor("uv_part", [TABROWS, 4], bf16)
    uv_red = nc.dram_tensor("uv_red", [TABROWS, 4], bf16)
    uv_tab = nc.dram_tensor("uv_tab", [TABROWS, P], bf16)

    with tile.TileContext(nc) as tc:
        with (
            tc.tile_pool(name="meta", bufs=1) as mp,
            tc.tile_pool(name="selp", bufs=16) as selp,
            tc.tile_pool(name="work", bufs=3) as wp,
            tc.tile_pool(name="psD", bufs=2, space="PSUM") as ppD,
        ):
            nc.gpsimd.load_library(mlp)
            iota_t = mp.tile([P, P], bf16, name="iota_t")
            nc.sync.dma_start(iota_t[:], iota_in[:])
            w1_sb = mp.tile([P, P], bf16, name="w1_sb")
            nc.sync.dma_start(w1_sb[:], w1t[:])
            wu_sb = mp.tile([P, 4], bf16, name="wu_sb")
            nc.sync.dma_start(wu_sb[:], wut[:])
            uv_full = mp.tile([P, GCHUNKS * 4], bf16, name="uv_full")

            def sel_build(name, rel_sb, w_sb, col, eng=None):
                sel = selp.tile([P, P], bf16, name=name, tag="sel")
                (eng or nc.vector).tensor_scalar(
                    out=sel[:], in0=iota_t[:],
                    scalar1=rel_sb[:, col:col + 1],
                    scalar2=w_sb[:, col:col + 1],
                    op0=mybir.AluOpType.is_equal,
                    op1=mybir.AluOpType.mult)
                return sel

            def make_stream(key, pr, tab_ap, ixp, sgp):
                rel_sb = ixp.tile([P, pr["n_sel"]], f32,
                                  name=f"rel_{key}_sb", tag=f"relt_{key}")
                w_sb = ixp.tile([P, pr["n_sel"]], f32,
                                name=f"w_{key}_sb", tag=f"wt_{key}")
                nc.sync.dma_start(rel_sb[:], relw_in[key][0][:])
                nc.sync.dma_start(w_sb[:], relw_in[key][1][:])
                idx_sb = []
                for r in range(4):
                    if (key, r) not in idx_in:
                        idx_sb.append(None)
                        continue
                    cols = pr["layout"][r]["T"] // 16
                    it = ixp.tile([P, cols], i16, name=f"ix_{key}_{r}",
                                  tag=f"ix_{key}_{r}")
                    nc.sync.dma_start(it[:], idx_in[(key, r)][:])
                    idx_sb.append(it)
                stage_tiles = {}

                def ensure_call(r, ci):
                    if (r, ci) in stage_tiles:
                        return stage_tiles[(r, ci)]
                    call = pr["layout"][r]["calls"][ci]
                    npad = call["n_pad"]
                    c0 = call["base"] // 16
                    st = sgp.tile([P, npad], bf16,
                                  name=f"st_{key}_{r}_{ci}",
                                  tag=f"st_{key}_{r}")
                    nc.gpsimd.dma_gather(
                        st[:].rearrange("p (c e) -> p c e", e=P),
                        tab_ap[RLO[r]:], idx_sb[r][:, c0:c0 + npad // 16],
                        npad, npad, P,
                        queue_num=0, single_packet=False)
                    stage_tiles[(r, ci)] = st
                    return st

                return dict(rel=rel_sb, w=w_sb, ensure=ensure_call)

            GRP = 4

            def run_layer1():
                with (
                    tc.tile_pool(name="ix_l1", bufs=1) as ixp,
                    tc.tile_pool(name="sg_l1", bufs=2) as sgp,
                    tc.tile_pool(name="psA", bufs=2, space="PSUM") as ppA,
                    tc.tile_pool(name="psB", bufs=2, space="PSUM") as ppB,
                ):
                    s = make_stream("l1", l1, xt, ixp, sgp)
                    nsel1 = [0]
                    for g in range((CHUNKS + GRP - 1) // GRP):
                        gs = min(GRP, CHUNKS - g * GRP)
                        psT = ppA.tile([P, gs * P], f32, space="PSUM",
                                       name=f"ps_l1_{g}", tag="psA")
                        for kk in range(gs):
                            k = g * GRP + kk
                            descs = l1["sched"][k]
                            for j, d in enumerate(descs):
                                st = s["ensure"](d["r"], d["call"])
                                nsel1[0] += 1
                                sel = sel_build(f"sel_l1_{k}_{j}", s["rel"],
                                                s["w"], d["sel"],
                                                eng=(nc.gpsimd
                                                     if nsel1[0] % 10 == 0
                                                     else None))
                                nc.tensor.matmul(
                                    psT[:, kk * P:(kk + 1) * P],
                                    lhsT=st[:, d["blk"] * P:
                                            (d["blk"] + 1) * P],
                                    rhs=sel[:],
                                    start=(j == 0),
                                    stop=(j == len(descs) - 1))
                        at = wp.tile([P, gs * P], bf16, name=f"at1_{g}",
                                     tag="at")
                        nc.scalar.copy(at[:], psT[:])
                        h_ps = ppB.tile([P, gs * P], f32, space="PSUM",
                                        name=f"h1_{g}", tag="psB")
                        for kk in range(gs):
                            nc.tensor.matmul(
                                h_ps[:, kk * P:(kk + 1) * P],
                                lhsT=at[:, kk * P:(kk + 1) * P],
                                rhs=w1_sb[:], start=True, stop=True)
                        h_sb = wp.tile([P, gs * P], bf16, name=f"h1s_{g}",
                                       tag="h")
                        nc.scalar.activation(
                            h_sb[:], h_ps[:],
                            mybir.ActivationFunctionType.Relu)
                        for kk in range(gs):
                            k = g * GRP + kk
                            nc.sync.dma_start(
                                h_slice[k * P:(k + 1) * P, :],
                                h_sb[:, kk * P:(kk + 1) * P])

            GRP2 = 8

            def run_layer2():
                with (
                    tc.tile_pool(name="ix_l2", bufs=1) as ixp,
                    tc.tile_pool(name="sg_l2", bufs=2) as sgp,
                    tc.tile_pool(name="psL2", bufs=2, space="PSUM") as ppL,
                    tc.tile_pool(name="psU2", bufs=2, space="PSUM") as ppU,
                ):
                    s = make_stream("l2", l2, h_slice[:], ixp, sgp)
                    nsel2 = [0]
                    for g in range(GCHUNKS // GRP2):
                        psT = ppL.tile([P, GRP2 * P], f32, space="PSUM",
                                       name=f"ps_l2_{g}", tag="psL")
                        for kk in range(GRP2):
                            k = g * GRP2 + kk
                            descs = l2["sched"][k]
                            for j, d in enumerate(descs):
                                st = s["ensure"](d["r"], d["call"])
                                nsel2[0] += 1
                                sel = sel_build(f"sel_l2_{k}_{j}", s["rel"],
                                                s["w"], d["sel"],
                                                eng=(nc.gpsimd
                                                     if nsel2[0] % 6 == 0
                                                     else None))
                                nc.tensor.matmul(
                                    psT[:, kk * P:(kk + 1) * P],
                                    lhsT=st[:, d["blk"] * P:
                                            (d["blk"] + 1) * P],
                                    rhs=sel[:],
                                    start=(j == 0),
                                    stop=(j == len(descs) - 1))
                        at = wp.tile([P, GRP2 * P], bf16, name=f"at2_{g}",
                                     tag="at2")
                        nc.scalar.copy(at[:], psT[:])
                        uv_ps = ppU.tile([P, GRP2 * 4], f32, space="PSUM",
                                         name=f"uvp_{g}", tag="psU")
                        for kk in range(GRP2):
                            nc.tensor.matmul(
                                uv_ps[:, kk * 4:(kk + 1) * 4],
                                lhsT=at[:, kk * P:(kk + 1) * P],
                                rhs=wu_sb[:], start=True, stop=True)
                        nc.scalar.copy(
                            uv_full[:, g * GRP2 * 4:(g + 1) * GRP2 * 4],
                            uv_ps[:])
                nc.sync.dma_start(uv_part[:], uv_full[:])

            run_layer1()
            run_layer2()
            # expand uv_red [TABROWS, 4] into zero-padded 256B-row uv_tab;
            # round-robin the big writes over 4 engines' DMA queues
            engs = [nc.sync, nc.scalar, nc.gpsimd, nc.sync]
            with tc.tile_pool(name="expp", bufs=1) as exp_p:
                bigs = []
                for b in range(4):
                    big = exp_p.tile([P, CHUNKS * P], bf16,
                                     name=f"exp_big_{b}")
                    nc.scalar.memzero(big[:])
                    bigs.append(big)
                nc.gpsimd.collective_compute(
                    "AllReduce", mybir.AluOpType.add,
                    replica_groups=[list(range(NCORES))],
                    ins=[uv_part[:]], outs=[uv_red[:]])
                for i in range(NCORES):
                    big = bigs[i % 4]
                    big3 = big[:].rearrange("p (g c) -> p g c", c=P)
                    nc.sync.dma_start(
                        big3[:, :, 0:4],
                        uv_red[i * SLOTS:(i + 1) * SLOTS, :])
                    engs[i % 4].dma_start(
                        uv_tab[i * SLOTS:(i + 1) * SLOTS, :], big[:])

            # decode: u and v streams into one [2, 128] psum per pair chunk
            with (
                tc.tile_pool(name="ix_dec", bufs=1) as ixd,
                tc.tile_pool(name="sg_dec", bufs=2) as sgd,
            ):
                su = make_stream("u", du, uv_tab[:], ixd, sgd)
                sv = make_stream("v", dv, uv_tab[:], ixd, sgd)
                osb = None
                for k in range(dchunks):
                    psD = ppD.tile([2, P], f32, space="PSUM",
                                   name=f"psd_{k}", tag="psD")
                    descs = ([(su, 0, d) for d in du["sched"][k]]
                             + [(sv, 2, d) for d in dv["sched"][k]])
                    for j, (s, off, d) in enumerate(descs):
                        st = s["ensure"](d["r"], d["call"])
                        sel = sel_build(f"sel_d{off}_{k}_{j}", s["rel"],
                                        s["w"], d["sel"],
                                        eng=(nc.gpsimd if j % 3 == 2
                                             else None))
                        c0 = d["blk"] * P + off
                        nc.tensor.matmul(
                            psD[:], lhsT=st[:, c0:c0 + 2], rhs=sel[:],
                            start=(j == 0), stop=(j == len(descs) - 1))
                    if k % 4 == 0:
                        osb = wp.tile([2, 4 * P], f32, name=f"osb_{k}",
                                      tag="osb")
                    nc.scalar.copy(osb[:, (k % 4) * P:(k % 4 + 1) * P],
                                   psD[:])
                    if k % 4 == 3:
                        nc.sync.dma_start(
                            out_d[:, (k - 3) * P:(k + 1) * P], osb[:])

    nc.compile()

    # ---------- stage inputs & run ----------
    iota_np = np.broadcast_to(np.arange(P, dtype=np.float32)[None, :],
                              (P, P)).astype(bf).copy()
    wcat_np = np.ascontiguousarray(
        np.concatenate([Wlin[:, :P].T, Wlin[:, P:].T], axis=1))
    wu_np = (W2 @ wcat_np).astype(bf)
    w1_np = W1.astype(bf)
    in_maps = []
    for c in range(NCORES):
        m = {"x_tab": x_tab, "W1r": w1_np, "Wu": wu_np, "iota": iota_np}
        for key, pr in (("l1", l1), ("l2", l2), ("u", du), ("v", dv)):
            m[f"rel_{key}"] = np.ascontiguousarray(pr["rel"][c])
            m[f"w_{key}"] = np.ascontiguousarray(pr["wgt"][c])
            for r in range(4):
                if (key, r) in idx_arr:
                    m[f"idx_{key}_{r}"] = idx_arr[(key, r)][c]
        in_maps.append(m)

    if globals().get("RUN_MODE", "hw") == "sim":
        results = _run_sim(nc, in_maps)
    else:
        res = run_bass_kernel_spmd(nc, in_maps, core_ids=list(range(NCORES)),
                                   trace=globals().get("TRACE", False))
        globals()["LAST_EXEC_NS"] = res.exec_time_ns
        results = res.results

    out = np.zeros((npairs, 2), np.float32)
    for c in range(NCORES):
        o2 = results[c]["out_dec"]                       # [2, per_core]
        m = (pair_slot >= c * per_core) & (pair_slot < (c + 1) * per_core)
        sl = pair_slot[m] - c * per_core
        out[m] = o2[:, sl].T
    return out


# revision 53
# speedup vs baseline: 1.0706x; 1.0706x over previous
"""GCN 2-layer + link decode on 8 TRN2 NeuronCores (full inputs in/out).

Design (dest-sharded, scatter-free, bf16):
- Aggregation commutes with the weight matmul: h = relu(segsum(w1*x[src]) @ W1);
  uv per node = segsum2(w2*h[src]) @ (W2 @ [WlinA.T|WlinB.T]) (4 cols).
- Each core owns 12544 destination slots. Edge streams sorted by
  (src int16-range, dest chunk, dest); bulk-gathered via dma_gather (int16
  local indices per 32768-row range, bf16 256B rows); routed+weighted into
  per-chunk PSUM by a selection-matrix matmul (swapped orientation:
  lhsT=staged rows, rhs=sel -> psum holds A^T[feat, slot], no transposes).
  sel[e,m] = (iota[e,m] == rel[e]) * w[e] built in ONE DVE tensor_scalar op
  (two scalar operands -> 4x_2p fast mode, bf16).
- Inter-layer exchange: each core writes its h/uv chunk slices directly into
  a Shared DRAM table via indirect-offset DMA (per-core row offsets are
  input data), then a tiny flag AllGather into scratch rows acts as the
  cross-core barrier (no 25-50MB AllGathers).
- Decode: pairs sorted by pos0, sharded by index; u and v streams routed
  into one [2, 128] psum per pair chunk (lhsT = uv cols 0:2 / 2:4);
  host unshuffles the [2, 25088] output.
"""
import numpy as np

P = 128
N = 100_000
NSHARD = 12_500
SLOTS = 12_544
CHUNKS = SLOTS // P          # 98
TABROWS = 8 * SLOTS          # 100352
SCR = 16                     # scratch rows for barrier AllGather
RBOUND = [32768, 65536, 98304]
RLO = [0, 32768, 65536, 98304]
NCORES = 8
CALL_CELLS = 8               # chunks per gather-call window


def _range_of(a):
    return np.searchsorted(RBOUND, a, side="right")


def _wrap_idx(a):
    """[NCORES, T] int16 -> [NCORES, 128, T//16] (16-wrap, 8x replicate)."""
    ncr, t = a.shape
    out = a.reshape(ncr, t // 16, 16).transpose(0, 2, 1)
    return np.ascontiguousarray(np.tile(out, (1, 8, 1)))


def _prep_stream(tab_row, slot, w, nchunks, call_cells):
    """Generic SPMD-uniform stream builder.

    tab_row: [E] global table row per entry; slot: [E] local out slot
    (0..nchunks*128); w: [E] weight; entries already per-core-filtered lists:
    tab_row etc are lists of arrays, one per core.
    Returns static schedule + per-core idx16 / rel / w arrays.
    """
    ncr = len(tab_row)
    # cell = (chunk, range); count per core
    counts = np.zeros((ncr, nchunks, 4), np.int64)
    for c in range(ncr):
        ch = slot[c] // P
        rr = _range_of(tab_row[c])
        np.add.at(counts, (c, ch, rr), 1)
    estar = counts.max(axis=0)                       # [nchunks, 4]

    # layout per range: calls of CALL windows, each padded to 128 multiple
    layout = []
    for r in range(4):
        if estar[:, r].sum() == 0:
            layout.append(dict(calls=[], T=0))
            continue
        calls = []
        base = 0
        for k0 in range(0, nchunks, call_cells):
            k1 = min(k0 + call_cells, nchunks)
            cells = estar[k0:k1, r]
            offs = np.concatenate([[0], np.cumsum(cells)]).astype(np.int64)
            n = int(offs[-1])
            n_pad = max(P, ((n + P - 1) // P) * P)
            calls.append(dict(k0=k0, k1=k1, offs=offs, n=n, n_pad=n_pad,
                              base=base))
            base += n_pad
        layout.append(dict(calls=calls, T=base))

    # static schedule: per chunk, matmul descriptors (r, call, blk, sel_col)
    sched = [[] for _ in range(nchunks)]
    selmap = {}
    n_sel = 0
    for r in range(4):
        for ci, call in enumerate(layout[r]["calls"]):
            nblk = call["n_pad"] // P
            offs, k0 = call["offs"], call["k0"]
            for b in range(nblk):
                e0, e1 = b * P, b * P + P
                ks = [k for k in range(call["k0"], call["k1"])
                      if offs[k - k0] < e1 and offs[k - k0 + 1] > e0]
                if not ks:
                    ks = [call["k0"]]
                for k in ks:
                    sched[k].append(dict(r=r, call=ci, blk=b, sel=n_sel))
                    selmap[(r, ci, b, k)] = n_sel
                    n_sel += 1

    idx16 = [np.zeros((ncr, layout[r]["T"]), np.int16) for r in range(4)]
    rel = np.zeros((ncr, P, n_sel), np.float32)
    wgt = np.zeros((ncr, P, n_sel), np.float32)

    for c in range(ncr):
        tr, sl, ww = tab_row[c], slot[c], w[c]
        rr = _range_of(tr)
        ch = sl // P
        # order entries by (range, chunk, slot)
        o = np.lexsort((sl, ch, rr))
        tr, sl, ww, rr, ch = tr[o], sl[o], ww[o], rr[o], ch[o]
        for r in range(4):
            m = rr == r
            if not m.any():
                continue
            trm, slm, wwm, chm = tr[m], sl[m], ww[m], ch[m]
            # position: call base + cell offset + within-cell rank
            cell_cnt = np.zeros(nchunks, np.int64)
            np.add.at(cell_cnt, chm, 1)
            cstart = np.concatenate([[0], np.cumsum(cell_cnt)])
            within = np.arange(len(slm)) - cstart[chm]
            call_id = chm // call_cells
            calls = layout[r]["calls"]
            cbase = np.array([cl["base"] for cl in calls], np.int64)
            # offset of chunk's cell within its call
            cell_off = np.zeros(nchunks, np.int64)
            for ci, cl in enumerate(calls):
                for k in range(cl["k0"], cl["k1"]):
                    cell_off[k] = cl["offs"][k - cl["k0"]]
            pos = cbase[call_id] + cell_off[chm] + within
            idx16[r][c, pos] = (trm - RLO[r]).astype(np.int16)
            # sel column per entry
            relpos = pos - cbase[call_id]
            blk = relpos // P
            pp = relpos % P
            cols = np.array([selmap[(r, int(ci_), int(b_), int(k_))]
                             for ci_, b_, k_ in zip(call_id, blk, chm)],
                            np.int64)
            rel[c, pp, cols] = (slm % P).astype(np.float32)
            wgt[c, pp, cols] = wwm
    return dict(layout=layout, sched=sched, n_sel=n_sel, idx16=idx16,
                rel=rel, wgt=wgt)


def _run_sim(nc, in_maps):
    """CoreSim timeline simulation: correctness + exec-time estimate."""
    import os
    from concourse.bass_interp import MultiCoreSim
    nc.insert_bir_kernel_barrier_sem_inc()
    sim = MultiCoreSim(nc, NCORES, aliases={},
                       require_finite=False, require_nnan=False,
                       trace=bool(os.environ.get("SIM_TRACE")),
                       num_workers=1)
    for c in range(NCORES):
        for name, arr in in_maps[c].items():
            sim.cores[c].tensor(name)[:] = arr
        if nc.partition_id_tensor is not None:
            sim.cores[c].tensor(nc.partition_id_tensor.name)[:] = np.array(
                [[c]], np.uint32)
    sim.simulate()
    globals()["LAST_EXEC_NS"] = int(sim.global_time)
    globals()["SIM_CORE_NS"] = [int(sim.cores[c].time) for c in range(NCORES)]
    if os.environ.get("SIM_DUMP"):
        np.save("/tmp/sim_h_slice.npy", np.stack(
            [np.asarray(sim.cores[c].tensor("h_slice"), np.float32)
             for c in range(NCORES)]))
        np.save("/tmp/sim_uv_red.npy",
                np.asarray(sim.cores[0].tensor("uv_red"), np.float32))
        np.save("/tmp/sim_uv_tab.npy",
                np.asarray(sim.cores[0].tensor("uv_tab"), np.float32))
        np.save("/tmp/sim_out.npy", np.stack(
            [np.asarray(sim.cores[c].tensor("out_dec"))
             for c in range(NCORES)]))
    return [{"out_dec": np.asarray(sim.cores[c].tensor("out_dec"))}
            for c in range(NCORES)]


def kernel(x, edge_index1, edge_index2, edge_weight1, edge_weight2,
           pos_edge_index, W1, W2, Wlin):
    import ml_dtypes
    import concourse.bass as bass
    from concourse import bacc, tile, mybir
    from concourse.bass_utils import run_bass_kernel_spmd
    from concourse.library_config import mlp

    f32, i16, i32 = mybir.dt.float32, mybir.dt.int16, mybir.dt.int32
    bf16 = mybir.dt.bfloat16
    bf = ml_dtypes.bfloat16
    x = np.asarray(x, np.float32)
    W1 = np.asarray(W1, np.float32)
    W2 = np.asarray(W2, np.float32)
    Wlin = np.asarray(Wlin, np.float32)
    e1 = np.asarray(edge_index1).astype(np.int64)
    e2 = np.asarray(edge_index2).astype(np.int64)
    w1 = np.asarray(edge_weight1, np.float32)
    w2 = np.asarray(edge_weight2, np.float32)
    pe = np.asarray(pos_edge_index).astype(np.int64)

    # ---------- host index preprocessing ----------
    x_tab = np.zeros((TABROWS, P), bf)
    x_tab[:N] = x.astype(bf)
    n2row = (np.arange(N) // NSHARD) * SLOTS + (np.arange(N) % NSHARD)

    def shard_by_dest(src_rows, dst, w):
        owner = dst // NSHARD
        ld = dst - owner * NSHARD
        return ([src_rows[owner == c] for c in range(NCORES)],
                [ld[owner == c] for c in range(NCORES)],
                [w[owner == c] for c in range(NCORES)])

    l1 = _prep_stream(*shard_by_dest(e1[0], e1[1], w1), CHUNKS, CALL_CELLS)

    # L2 sharded by SOURCE owner: each core gathers only from its own
    # h_slice; dest slots are global (784 chunks over the n2row space).
    GCHUNKS = TABROWS // P                               # 784
    owner2 = e2[0] // NSHARD
    l2 = _prep_stream(
        [e2[0][owner2 == c] - c * NSHARD for c in range(NCORES)],
        [n2row[e2[1]][owner2 == c] for c in range(NCORES)],
        [w2[owner2 == c] for c in range(NCORES)],
        GCHUNKS, CALL_CELLS)

    # uv tables live in a transposed row order: n2row j -> (j%128)*784+j//128
    def sig(j):
        return (j % P) * GCHUNKS + j // P

    # decode: shard pairs by original index; per core sort by pos0-row and
    # use the local sorted position as the output slot (host unshuffles).
    t0 = sig(n2row[pe[0]])
    t1 = sig(n2row[pe[1]])
    npairs = pe.shape[1]
    pershard = (npairs + NCORES - 1) // NCORES            # 25000
    per_core = ((pershard + P - 1) // P) * P              # 25088
    dchunks = per_core // P
    pair_slot = np.empty(npairs, np.int64)
    u_rows, u_slots, u_w = [], [], []
    v_rows, v_slots, v_w = [], [], []
    for c in range(NCORES):
        p0, p1 = c * pershard, min((c + 1) * pershard, npairs)
        loc = np.argsort(t0[p0:p1], kind="stable")
        sl = np.empty(p1 - p0, np.int64)
        sl[loc] = np.arange(p1 - p0)
        pair_slot[p0:p1] = c * per_core + sl
        ones = np.ones(p1 - p0, np.float32)
        u_rows.append(t0[p0:p1]); u_slots.append(sl); u_w.append(ones)
        v_rows.append(t1[p0:p1]); v_slots.append(sl); v_w.append(ones)
    du = _prep_stream(u_rows, u_slots, u_w, dchunks, 48)
    dv = _prep_stream(v_rows, v_slots, v_w, dchunks, 48)

    idx_arr = {}
    for key, pr in (("l1", l1), ("l2", l2), ("u", du), ("v", dv)):
        for r in range(4):
            if pr["layout"][r]["T"] == 0:
                continue
            idx_arr[(key, r)] = _wrap_idx(pr["idx16"][r])

    # ---------- device program ----------
    nc = bacc.Bacc("TRN2", target_bir_lowering=False, debug=False,
                   num_devices=NCORES, num_swdge_queues=4)

    def din(name, shape, dt=f32):
        return nc.dram_tensor(name, list(shape), dt, kind="ExternalInput").ap()

    xt = din("x_tab", (TABROWS, P), bf16)
    w1t = din("W1r", (P, P), bf16)
    wut = din("Wu", (P, 4), bf16)
    iota_in = din("iota", (P, P), bf16)
    idx_in = {k: din(f"idx_{k[0]}_{k[1]}", v.shape[1:], i16)
              for k, v in idx_arr.items()}
    relw_in = {key: (din(f"rel_{key}", (P, pr["n_sel"])),
                     din(f"w_{key}", (P, pr["n_sel"])))
               for key, pr in (("l1", l1), ("l2", l2), ("u", du), ("v", dv))}

    out_d = nc.dram_tensor("out_dec", [2, per_core], f32,
                           kind="ExternalOutput").ap()
    h_slice = nc.dram_tensor("h_slice", [SLOTS, P], bf16)
    uv_part = nc.dram_tensor("uv_part", [TABROWS, 4], bf16)
    uv_rs = nc.dram_tensor("uv_rs", [SLOTS, 4], bf16)
    uv_red = nc.dram_tensor("uv_red", [TABROWS, 4], bf16)
    uv_tab = nc.dram_tensor("uv_tab", [TABROWS, P], bf16)

    with tile.TileContext(nc) as tc:
        with (
            tc.tile_pool(name="meta", bufs=1) as mp,
            tc.tile_pool(name="selp", bufs=16) as selp,
            tc.tile_pool(name="work", bufs=3) as wp,
        ):
            nc.gpsimd.load_library(mlp)
            iota_t = mp.tile([P, P], bf16, name="iota_t")
            nc.sync.dma_start(iota_t[:], iota_in[:])
            w1_sb = mp.tile([P, P], bf16, name="w1_sb")
            nc.sync.dma_start(w1_sb[:], w1t[:])
            wu_sb = mp.tile([P, 4], bf16, name="wu_sb")
            nc.sync.dma_start(wu_sb[:], wut[:])
            uv_full = mp.tile([P, GCHUNKS * 4], bf16, name="uv_full")

            def sel_build(name, rel_sb, w_sb, col, eng=None):
                sel = selp.tile([P, P], bf16, name=name, tag="sel")
                (eng or nc.vector).tensor_scalar(
                    out=sel[:], in0=iota_t[:],
                    scalar1=rel_sb[:, col:col + 1],
                    scalar2=w_sb[:, col:col + 1],
                    op0=mybir.AluOpType.is_equal,
                    op1=mybir.AluOpType.mult)
                return sel

            def make_stream(key, pr, tab_ap, ixp, sgp):
                rel_sb = ixp.tile([P, pr["n_sel"]], f32,
                                  name=f"rel_{key}_sb", tag=f"relt_{key}")
                w_sb = ixp.tile([P, pr["n_sel"]], f32,
                                name=f"w_{key}_sb", tag=f"wt_{key}")
                nc.sync.dma_start(rel_sb[:], relw_in[key][0][:])
                nc.sync.dma_start(w_sb[:], relw_in[key][1][:])
                idx_sb = []
                for r in range(4):
                    if (key, r) not in idx_in:
                        idx_sb.append(None)
                        continue
                    cols = pr["layout"][r]["T"] // 16
                    it = ixp.tile([P, cols], i16, name=f"ix_{key}_{r}",
                                  tag=f"ix_{key}_{r}")
                    nc.sync.dma_start(it[:], idx_in[(key, r)][:])
                    idx_sb.append(it)
                stage_tiles = {}

                def ensure_call(r, ci):
                    if (r, ci) in stage_tiles:
                        return stage_tiles[(r, ci)]
                    call = pr["layout"][r]["calls"][ci]
                    npad = call["n_pad"]
                    c0 = call["base"] // 16
                    st = sgp.tile([P, npad], bf16,
                                  name=f"st_{key}_{r}_{ci}",
                                  tag=f"st_{key}_{r}")
                    nc.gpsimd.dma_gather(
                        st[:].rearrange("p (c e) -> p c e", e=P),
                        tab_ap[RLO[r]:], idx_sb[r][:, c0:c0 + npad // 16],
                        npad, npad, P,
                        queue_num=0, single_packet=False)
                    stage_tiles[(r, ci)] = st
                    return st

                return dict(rel=rel_sb, w=w_sb, ensure=ensure_call)

            GRP = 4

            def run_layer1():
                with (
                    tc.tile_pool(name="ix_l1", bufs=1) as ixp,
                    tc.tile_pool(name="sg_l1", bufs=3) as sgp,
                    tc.tile_pool(name="psA", bufs=3, space="PSUM") as ppA,
                    tc.tile_pool(name="psB", bufs=2, space="PSUM") as ppB,
                ):
                    s = make_stream("l1", l1, xt, ixp, sgp)
                    nsel1 = [0]
                    for g in range((CHUNKS + GRP - 1) // GRP):
                        gs = min(GRP, CHUNKS - g * GRP)
                        psT = ppA.tile([P, gs * P], f32, space="PSUM",
                                       name=f"ps_l1_{g}", tag="psA")
                        for kk in range(gs):
                            k = g * GRP + kk
                            descs = l1["sched"][k]
                            for j, d in enumerate(descs):
                                st = s["ensure"](d["r"], d["call"])
                                nsel1[0] += 1
                                sel = sel_build(f"sel_l1_{k}_{j}", s["rel"],
                                                s["w"], d["sel"])
                                nc.tensor.matmul(
                                    psT[:, kk * P:(kk + 1) * P],
                                    lhsT=st[:, d["blk"] * P:
                                            (d["blk"] + 1) * P],
                                    rhs=sel[:],
                                    start=(j == 0),
                                    stop=(j == len(descs) - 1))
                        at = wp.tile([P, gs * P], bf16, name=f"at1_{g}",
                                     tag="at")
                        nc.scalar.copy(at[:], psT[:])
                        h_ps = ppB.tile([P, gs * P], f32, space="PSUM",
                                        name=f"h1_{g}", tag="psB")
                        for kk in range(gs):
                            nc.tensor.matmul(
                                h_ps[:, kk * P:(kk + 1) * P],
                                lhsT=at[:, kk * P:(kk + 1) * P],
                                rhs=w1_sb[:], start=True, stop=True)
                        h_sb = wp.tile([P, gs * P], bf16, name=f"h1s_{g}",
                                       tag="h")
                        nc.scalar.activation(
                            h_sb[:], h_ps[:],
                            mybir.ActivationFunctionType.Relu)
                        for kk in range(gs):
                            k = g * GRP + kk
                            nc.sync.dma_start(
                                h_slice[k * P:(k + 1) * P, :],
                                h_sb[:, kk * P:(kk + 1) * P])

            GRP2 = 8

            def run_layer2():
                with (
                    tc.tile_pool(name="ix_l2", bufs=1) as ixp,
                    tc.tile_pool(name="sg_l2", bufs=3) as sgp,
                    tc.tile_pool(name="psL2", bufs=3, space="PSUM") as ppL,
                    tc.tile_pool(name="psU2", bufs=2, space="PSUM") as ppU,
                ):
                    s = make_stream("l2", l2, h_slice[:], ixp, sgp)
                    nsel2 = [0]
                    for g in range(GCHUNKS // GRP2):
                        psT = ppL.tile([P, GRP2 * P], f32, space="PSUM",
                                       name=f"ps_l2_{g}", tag="psL")
                        for kk in range(GRP2):
                            k = g * GRP2 + kk
                            descs = l2["sched"][k]
                            for j, d in enumerate(descs):
                                st = s["ensure"](d["r"], d["call"])
                                nsel2[0] += 1
                                sel = sel_build(f"sel_l2_{k}_{j}", s["rel"],
                                                s["w"], d["sel"],
                                                eng=(nc.gpsimd
                                                     if nsel2[0] % 6 == 0
                                                     else None))
                                nc.tensor.matmul(
                                    psT[:, kk * P:(kk + 1) * P],
                                    lhsT=st[:, d["blk"] * P:
                                            (d["blk"] + 1) * P],
                                    rhs=sel[:],
                                    start=(j == 0),
                                    stop=(j == len(descs) - 1))
                        at = wp.tile([P, GRP2 * P], bf16, name=f"at2_{g}",
                                     tag="at2")
                        nc.scalar.copy(at[:], psT[:])
                        uv_ps = ppU.tile([P, GRP2 * 4], f32, space="PSUM",
                                         name=f"uvp_{g}", tag="psU")
                        for kk in range(GRP2):
                            nc.tensor.matmul(
                                uv_ps[:, kk * 4:(kk + 1) * 4],
                                lhsT=at[:, kk * P:(kk + 1) * P],
                                rhs=wu_sb[:], start=True, stop=True)
                        nc.scalar.copy(
                            uv_full[:, g * GRP2 * 4:(g + 1) * GRP2 * 4],
                            uv_ps[:])
                nc.sync.dma_start(uv_part[:], uv_full[:])

            run_layer1()
            run_layer2()
            # expand uv_red [TABROWS, 4] into zero-padded 256B-row uv_tab;
            # round-robin the big writes over 4 engines' DMA queues
            engs = [nc.sync, nc.scalar, nc.gpsimd, nc.sync]
            with tc.tile_pool(name="expp", bufs=1) as exp_p:
                bigs = []
                for b in range(4):
                    big = exp_p.tile([P, CHUNKS * P], bf16,
                                     name=f"exp_big_{b}")
                    nc.scalar.memzero(big[:])
                    bigs.append(big)
                nc.gpsimd.collective_compute(
                    "ReduceScatter", mybir.AluOpType.add,
                    replica_groups=[list(range(NCORES))],
                    ins=[uv_part[:]], outs=[uv_rs[:]])
                nc.gpsimd.collective_compute(
                    "AllGather", mybir.AluOpType.bypass,
                    replica_groups=[list(range(NCORES))],
                    ins=[uv_rs[:]], outs=[uv_red[:]])
                for i in range(NCORES):
                    big = bigs[i % 4]
                    big3 = big[:].rearrange("p (g c) -> p g c", c=P)
                    nc.sync.dma_start(
                        big3[:, :, 0:4],
                        uv_red[i * SLOTS:(i + 1) * SLOTS, :])
                    engs[i % 4].dma_start(
                        uv_tab[i * SLOTS:(i + 1) * SLOTS, :], big[:])

            # decode: u and v streams into one [2, 128] psum per pair chunk
            with (
                tc.tile_pool(name="psD", bufs=2, space="PSUM") as ppD,
                tc.tile_pool(name="ix_dec", bufs=1) as ixd,
                tc.tile_pool(name="sg_dec", bufs=2) as sgd,
            ):
                su = make_stream("u", du, uv_tab[:], ixd, sgd)
                sv = make_stream("v", dv, uv_tab[:], ixd, sgd)
                osb = None
                for k in range(dchunks):
                    psD = ppD.tile([2, P], f32, space="PSUM",
                                   name=f"psd_{k}", tag="psD")
                    descs = ([(su, 0, d) for d in du["sched"][k]]
                             + [(sv, 2, d) for d in dv["sched"][k]])
                    for j, (s, off, d) in enumerate(descs):
                        st = s["ensure"](d["r"], d["call"])
                        sel = sel_build(f"sel_d{off}_{k}_{j}", s["rel"],
                                        s["w"], d["sel"],
                                        eng=(nc.gpsimd if j % 4 == 3
                                             else None))
                        c0 = d["blk"] * P + off
                        nc.tensor.matmul(
                            psD[:], lhsT=st[:, c0:c0 + 2], rhs=sel[:],
                            start=(j == 0), stop=(j == len(descs) - 1))
                    if k % 4 == 0:
                        osb = wp.tile([2, 4 * P], f32, name=f"osb_{k}",
                                      tag="osb")
                    nc.scalar.copy(osb[:, (k % 4) * P:(k % 4 + 1) * P],
                                   psD[:])
                    if k % 4 == 3:
                        nc.sync.dma_start(
                            out_d[:, (k - 3) * P:(k + 1) * P], osb[:])

    nc.compile()

    # ---------- stage inputs & run ----------
    iota_np = np.broadcast_to(np.arange(P, dtype=np.float32)[None, :],
                              (P, P)).astype(bf).copy()
    wcat_np = np.ascontiguousarray(
        np.concatenate([Wlin[:, :P].T, Wlin[:, P:].T], axis=1))
    wu_np = (W2 @ wcat_np).astype(bf)
    w1_np = W1.astype(bf)
    in_maps = []
    for c in range(NCORES):
        m = {"x_tab": x_tab, "W1r": w1_np, "Wu": wu_np, "iota": iota_np}
        for key, pr in (("l1", l1), ("l2", l2), ("u", du), ("v", dv)):
            m[f"rel_{key}"] = np.ascontiguousarray(pr["rel"][c])
            m[f"w_{key}"] = np.ascontiguousarray(pr["wgt"][c])
            for r in range(4):
                if (key, r) in idx_arr:
                    m[f"idx_{key}_{r}"] = idx_arr[(key, r)][c]
        in_maps.append(m)

    if globals().get("RUN_MODE", "hw") == "sim":
        results = _run_sim(nc, in_maps)
    else:
        res = run_bass_kernel_spmd(nc, in_maps, core_ids=list(range(NCORES)),
                                   trace=globals().get("TRACE", False))
        globals()["LAST_EXEC_NS"] = res.exec_time_ns
        results = res.results

    out = np.zeros((npairs, 2), np.float32)
    for c in range(NCORES):
        o2 = results[c]["out_dec"]                       # [2, per_core]
        m = (pair_slot >= c * per_core) & (pair_slot < (c + 1) * per_core)
        sl = pair_slot[m] - c * per_core
        out[m] = o2[:, sl].T
    return out


# revision 69
# speedup vs baseline: 1.1873x; 1.1090x over previous
"""GCN 2-layer + link decode on 8 TRN2 NeuronCores (full inputs in/out).

Design (dest-sharded, scatter-free, bf16):
- Aggregation commutes with the weight matmul: h = relu(segsum(w1*x[src]) @ W1);
  uv per node = segsum2(w2*h[src]) @ (W2 @ [WlinA.T|WlinB.T]) (4 cols).
- Each core owns 12544 destination slots. Edge streams sorted by
  (src int16-range, dest chunk, dest); bulk-gathered via dma_gather (int16
  local indices per 32768-row range, bf16 256B rows); routed+weighted into
  per-chunk PSUM by a selection-matrix matmul (swapped orientation:
  lhsT=staged rows, rhs=sel -> psum holds A^T[feat, slot], no transposes).
  sel[e,m] = (iota[e,m] == rel[e]) * w[e] built in ONE DVE tensor_scalar op
  (two scalar operands -> 4x_2p fast mode, bf16).
- Inter-layer exchange: each core writes its h/uv chunk slices directly into
  a Shared DRAM table via indirect-offset DMA (per-core row offsets are
  input data), then a tiny flag AllGather into scratch rows acts as the
  cross-core barrier (no 25-50MB AllGathers).
- Decode: pairs sorted by pos0, sharded by index; u and v streams routed
  into one [2, 128] psum per pair chunk (lhsT = uv cols 0:2 / 2:4);
  host unshuffles the [2, 25088] output.
"""
import numpy as np

P = 128
N = 100_000
NSHARD = 12_500
SLOTS = 12_544
CHUNKS = SLOTS // P          # 98
TABROWS = 8 * SLOTS          # 100352
SCR = 16                     # scratch rows for barrier AllGather
RBOUND = [32768, 65536, 98304]
RLO = [0, 32768, 65536, 98304]
NCORES = 8
CALL_CELLS = 8               # chunks per gather-call window


def _range_of(a):
    return np.searchsorted(RBOUND, a, side="right")


def _wrap_idx(a):
    """[NCORES, T] int16 -> [NCORES, 128, T//16] (16-wrap, 8x replicate)."""
    ncr, t = a.shape
    out = a.reshape(ncr, t // 16, 16).transpose(0, 2, 1)
    return np.ascontiguousarray(np.tile(out, (1, 8, 1)))


MAXNK = 2  # max psum windows (128 cols each) one merged sel may span;
           # capped at 2 because iota/rel are bf16 (integers exact to 256)


def _prep_stream(tab_row, slot, w, nchunks, call_cells, grp):
    """Generic SPMD-uniform stream builder.

    tab_row: [E] global table row per entry; slot: [E] local out slot
    (0..nchunks*128); w: [E] weight; entries already per-core-filtered lists:
    tab_row etc are lists of arrays, one per core. grp = psum-group size in
    chunks; descs spanning consecutive chunks within one group are merged
    into a single wider sel (rel is relative to the desc's first chunk).
    Returns static schedule (per GROUP) + per-core idx16 / rel / w arrays.
    """
    ncr = len(tab_row)
    # cell = (chunk, range); count per core
    counts = np.zeros((ncr, nchunks, 4), np.int64)
    for c in range(ncr):
        ch = slot[c] // P
        rr = _range_of(tab_row[c])
        np.add.at(counts, (c, ch, rr), 1)
    estar = counts.max(axis=0)                       # [nchunks, 4]

    # layout per range: calls of CALL windows, each padded to 128 multiple
    layout = []
    for r in range(4):
        if estar[:, r].sum() == 0:
            layout.append(dict(calls=[], T=0))
            continue
        calls = []
        base = 0
        for k0 in range(0, nchunks, call_cells):
            k1 = min(k0 + call_cells, nchunks)
            cells = estar[k0:k1, r]
            offs = np.concatenate([[0], np.cumsum(cells)]).astype(np.int64)
            n = int(offs[-1])
            n_pad = max(P, ((n + P - 1) // P) * P)
            calls.append(dict(k0=k0, k1=k1, offs=offs, n=n, n_pad=n_pad,
                              base=base))
            base += n_pad
        layout.append(dict(calls=calls, T=base))

    # static schedule: per psum-GROUP, matmul descriptors
    # (r, call, blk, sel_col, k0, nk)
    sched = [[] for _ in range((nchunks + grp - 1) // grp)]
    selmap = {}
    n_sel = 0
    for r in range(4):
        for ci, call in enumerate(layout[r]["calls"]):
            nblk = call["n_pad"] // P
            offs, k0c = call["offs"], call["k0"]
            for b in range(nblk):
                e0, e1 = b * P, b * P + P
                ks = [k for k in range(call["k0"], call["k1"])
                      if offs[k - k0c] < e1 and offs[k - k0c + 1] > e0]
                if not ks:
                    ks = [call["k0"]]
                # merged matmul windows must stay inside one PSUM bank
                # (512 f32 = 4 chunks) — and therefore inside one group
                runs = [[ks[0]]]
                for k in ks[1:]:
                    if (k == runs[-1][-1] + 1
                            and k // 4 == runs[-1][0] // 4
                            and len(runs[-1]) < MAXNK):
                        runs[-1].append(k)
                    else:
                        runs.append([k])
                for run in runs:
                    sched[run[0] // grp].append(
                        dict(r=r, call=ci, blk=b, sel=n_sel,
                             k0=run[0], nk=len(run)))
                    for k in run:
                        selmap[(r, ci, b, k)] = (n_sel, run[0])
                    n_sel += 1

    idx16 = [np.zeros((ncr, layout[r]["T"]), np.int16) for r in range(4)]
    rel = np.zeros((ncr, P, n_sel), np.float32)
    wgt = np.zeros((ncr, P, n_sel), np.float32)

    for c in range(ncr):
        tr, sl, ww = tab_row[c], slot[c], w[c]
        rr = _range_of(tr)
        ch = sl // P
        # order entries by (range, chunk, slot)
        o = np.lexsort((sl, ch, rr))
        tr, sl, ww, rr, ch = tr[o], sl[o], ww[o], rr[o], ch[o]
        for r in range(4):
            m = rr == r
            if not m.any():
                continue
            trm, slm, wwm, chm = tr[m], sl[m], ww[m], ch[m]
            # position: call base + cell offset + within-cell rank
            cell_cnt = np.zeros(nchunks, np.int64)
            np.add.at(cell_cnt, chm, 1)
            cstart = np.concatenate([[0], np.cumsum(cell_cnt)])
            within = np.arange(len(slm)) - cstart[chm]
            call_id = chm // call_cells
            calls = layout[r]["calls"]
            cbase = np.array([cl["base"] for cl in calls], np.int64)
            # offset of chunk's cell within its call
            cell_off = np.zeros(nchunks, np.int64)
            for ci, cl in enumerate(calls):
                for k in range(cl["k0"], cl["k1"]):
                    cell_off[k] = cl["offs"][k - cl["k0"]]
            pos = cbase[call_id] + cell_off[chm] + within
            idx16[r][c, pos] = (trm - RLO[r]).astype(np.int16)
            # sel column per entry (rel is relative to the desc's k0 chunk)
            relpos = pos - cbase[call_id]
            blk = relpos // P
            pp = relpos % P
            pairs = [selmap[(r, int(ci_), int(b_), int(k_))]
                     for ci_, b_, k_ in zip(call_id, blk, chm)]
            cols = np.array([p[0] for p in pairs], np.int64)
            k0s = np.array([p[1] for p in pairs], np.int64)
            rel[c, pp, cols] = (slm - k0s * P).astype(np.float32)
            wgt[c, pp, cols] = wwm
    return dict(layout=layout, sched=sched, n_sel=n_sel, idx16=idx16,
                rel=rel, wgt=wgt)


def _run_sim(nc, in_maps):
    """CoreSim timeline simulation: correctness + exec-time estimate."""
    import os
    from concourse.bass_interp import MultiCoreSim
    nc.insert_bir_kernel_barrier_sem_inc()
    sim = MultiCoreSim(nc, NCORES, aliases={},
                       require_finite=False, require_nnan=False,
                       trace=bool(os.environ.get("SIM_TRACE")),
                       num_workers=1)
    for c in range(NCORES):
        for name, arr in in_maps[c].items():
            sim.cores[c].tensor(name)[:] = arr
        if nc.partition_id_tensor is not None:
            sim.cores[c].tensor(nc.partition_id_tensor.name)[:] = np.array(
                [[c]], np.uint32)
    sim.simulate()
    globals()["LAST_EXEC_NS"] = int(sim.global_time)
    globals()["SIM_CORE_NS"] = [int(sim.cores[c].time) for c in range(NCORES)]
    if os.environ.get("SIM_DUMP"):
        np.save("/tmp/sim_h_slice.npy", np.stack(
            [np.asarray(sim.cores[c].tensor("h_slice"), np.float32)
             for c in range(NCORES)]))
        np.save("/tmp/sim_uv_red.npy",
                np.asarray(sim.cores[0].tensor("uv_red"), np.float32))
        np.save("/tmp/sim_uv_tab.npy",
                np.asarray(sim.cores[0].tensor("uv_tab"), np.float32))
        np.save("/tmp/sim_out.npy", np.stack(
            [np.asarray(sim.cores[c].tensor("out_dec"))
             for c in range(NCORES)]))
    return [{"out_dec": np.asarray(sim.cores[c].tensor("out_dec"))}
            for c in range(NCORES)]


def kernel(x, edge_index1, edge_index2, edge_weight1, edge_weight2,
           pos_edge_index, W1, W2, Wlin):
    import ml_dtypes
    import concourse.bass as bass
    from concourse import bacc, tile, mybir
    from concourse.bass_utils import run_bass_kernel_spmd
    from concourse.library_config import mlp

    f32, i16, i32 = mybir.dt.float32, mybir.dt.int16, mybir.dt.int32
    bf16 = mybir.dt.bfloat16
    bf = ml_dtypes.bfloat16
    x = np.asarray(x, np.float32)
    W1 = np.asarray(W1, np.float32)
    W2 = np.asarray(W2, np.float32)
    Wlin = np.asarray(Wlin, np.float32)
    e1 = np.asarray(edge_index1).astype(np.int64)
    e2 = np.asarray(edge_index2).astype(np.int64)
    w1 = np.asarray(edge_weight1, np.float32)
    w2 = np.asarray(edge_weight2, np.float32)
    pe = np.asarray(pos_edge_index).astype(np.int64)

    # ---------- host index preprocessing ----------
    x_tab = np.zeros((TABROWS, P), bf)
    x_tab[:N] = x.astype(bf)
    n2row = (np.arange(N) // NSHARD) * SLOTS + (np.arange(N) % NSHARD)

    def shard_by_dest(src_rows, dst, w):
        owner = dst // NSHARD
        ld = dst - owner * NSHARD
        return ([src_rows[owner == c] for c in range(NCORES)],
                [ld[owner == c] for c in range(NCORES)],
                [w[owner == c] for c in range(NCORES)])

    l1 = _prep_stream(*shard_by_dest(e1[0], e1[1], w1), CHUNKS, CALL_CELLS, 4)

    # L2 sharded by SOURCE owner: each core gathers only from its own
    # h_slice; dest slots are global (784 chunks over the n2row space).
    GCHUNKS = TABROWS // P                               # 784
    owner2 = e2[0] // NSHARD
    l2 = _prep_stream(
        [e2[0][owner2 == c] - c * NSHARD for c in range(NCORES)],
        [n2row[e2[1]][owner2 == c] for c in range(NCORES)],
        [w2[owner2 == c] for c in range(NCORES)],
        GCHUNKS, CALL_CELLS, 8)

    # uv tables live in a transposed row order: n2row j -> (j%128)*784+j//128
    def sig(j):
        return (j % P) * GCHUNKS + j // P

    # decode: shard pairs by original index; per core sort by pos0-row and
    # use the local sorted position as the output slot (host unshuffles).
    t0 = sig(n2row[pe[0]])
    t1 = sig(n2row[pe[1]])
    npairs = pe.shape[1]
    pershard = (npairs + NCORES - 1) // NCORES            # 25000
    per_core = ((pershard + P - 1) // P) * P              # 25088
    dchunks = per_core // P
    pair_slot = np.empty(npairs, np.int64)
    u_rows, u_slots, u_w = [], [], []
    v_rows, v_slots, v_w = [], [], []
    for c in range(NCORES):
        p0, p1 = c * pershard, min((c + 1) * pershard, npairs)
        loc = np.argsort(t0[p0:p1], kind="stable")
        sl = np.empty(p1 - p0, np.int64)
        sl[loc] = np.arange(p1 - p0)
        pair_slot[p0:p1] = c * per_core + sl
        ones = np.ones(p1 - p0, np.float32)
        u_rows.append(t0[p0:p1]); u_slots.append(sl); u_w.append(ones)
        v_rows.append(t1[p0:p1]); v_slots.append(sl); v_w.append(ones)
    du = _prep_stream(u_rows, u_slots, u_w, dchunks, 48, 4)
    dv = _prep_stream(v_rows, v_slots, v_w, dchunks, 48, 4)

    idx_arr = {}
    for key, pr in (("l1", l1), ("l2", l2), ("u", du), ("v", dv)):
        for r in range(4):
            if pr["layout"][r]["T"] == 0:
                continue
            idx_arr[(key, r)] = _wrap_idx(pr["idx16"][r])

    # ---------- device program ----------
    nc = bacc.Bacc("TRN2", target_bir_lowering=False, debug=False,
                   num_devices=NCORES, num_swdge_queues=4)

    def din(name, shape, dt=f32):
        return nc.dram_tensor(name, list(shape), dt, kind="ExternalInput").ap()

    xt = din("x_tab", (TABROWS, P), bf16)
    w1t = din("W1r", (P, P), bf16)
    wut = din("Wu", (P, 4), bf16)
    iota_in = din("iota", (P, MAXNK * P), bf16)
    idx_in = {k: din(f"idx_{k[0]}_{k[1]}", v.shape[1:], i16)
              for k, v in idx_arr.items()}
    relw_in = {key: (din(f"rel_{key}", (P, pr["n_sel"])),
                     din(f"w_{key}", (P, pr["n_sel"])))
               for key, pr in (("l1", l1), ("l2", l2), ("u", du), ("v", dv))}

    out_d = nc.dram_tensor("out_dec", [2, per_core], f32,
                           kind="ExternalOutput").ap()
    h_slice = nc.dram_tensor("h_slice", [SLOTS, P], bf16)
    uv_part = nc.dram_tensor("uv_part", [TABROWS, 4], bf16)
    uv_rs = nc.dram_tensor("uv_rs", [SLOTS, 4], bf16)
    uv_red = nc.dram_tensor("uv_red", [TABROWS, 4], bf16)
    uv_tab = nc.dram_tensor("uv_tab", [TABROWS, P], bf16)

    with tile.TileContext(nc) as tc:
        with (
            tc.tile_pool(name="meta", bufs=1) as mp,
            tc.tile_pool(name="selp", bufs=16) as selp,
            tc.tile_pool(name="work", bufs=3) as wp,
        ):
            nc.gpsimd.load_library(mlp)
            iota_t = mp.tile([P, MAXNK * P], bf16, name="iota_t")
            nc.sync.dma_start(iota_t[:], iota_in[:])
            zeros_t = mp.tile([P, 8 * P], bf16, name="zeros_t")
            nc.vector.memset(zeros_t[:], 0.0)
            w1_sb = mp.tile([P, P], bf16, name="w1_sb")
            nc.sync.dma_start(w1_sb[:], w1t[:])
            wu_sb = mp.tile([P, 4], bf16, name="wu_sb")
            nc.sync.dma_start(wu_sb[:], wut[:])
            uv_full = mp.tile([P, GCHUNKS * 4], bf16, name="uv_full")

            def sel_build(name, rel_sb, w_sb, col, w=P, eng=None):
                sel = selp.tile([P, w], bf16, name=name, tag="sel")
                (eng or nc.vector).tensor_scalar(
                    out=sel[:], in0=iota_t[:, :w],
                    scalar1=rel_sb[:, col:col + 1],
                    scalar2=w_sb[:, col:col + 1],
                    op0=mybir.AluOpType.is_equal,
                    op1=mybir.AluOpType.mult)
                return sel

            def make_stream(key, pr, tab_ap, ixp, sgp):
                rel_sb = ixp.tile([P, pr["n_sel"]], f32,
                                  name=f"rel_{key}_sb", tag=f"relt_{key}")
                w_sb = ixp.tile([P, pr["n_sel"]], f32,
                                name=f"w_{key}_sb", tag=f"wt_{key}")
                nc.sync.dma_start(rel_sb[:], relw_in[key][0][:])
                nc.sync.dma_start(w_sb[:], relw_in[key][1][:])
                idx_sb = []
                for r in range(4):
                    if (key, r) not in idx_in:
                        idx_sb.append(None)
                        continue
                    cols = pr["layout"][r]["T"] // 16
                    it = ixp.tile([P, cols], i16, name=f"ix_{key}_{r}",
                                  tag=f"ix_{key}_{r}")
                    nc.sync.dma_start(it[:], idx_in[(key, r)][:])
                    idx_sb.append(it)
                stage_tiles = {}

                def ensure_call(r, ci):
                    if (r, ci) in stage_tiles:
                        return stage_tiles[(r, ci)]
                    call = pr["layout"][r]["calls"][ci]
                    npad = call["n_pad"]
                    c0 = call["base"] // 16
                    st = sgp.tile([P, npad], bf16,
                                  name=f"st_{key}_{r}_{ci}",
                                  tag=f"st_{key}_{r}")
                    nc.gpsimd.dma_gather(
                        st[:].rearrange("p (c e) -> p c e", e=P),
                        tab_ap[RLO[r]:], idx_sb[r][:, c0:c0 + npad // 16],
                        npad, npad, P,
                        queue_num=0, single_packet=False)
                    stage_tiles[(r, ci)] = st
                    return st

                return dict(rel=rel_sb, w=w_sb, ensure=ensure_call)

            GRP = 4

            def run_layer1():
                with (
                    tc.tile_pool(name="ix_l1", bufs=1) as ixp,
                    tc.tile_pool(name="sg_l1", bufs=3) as sgp,
                    tc.tile_pool(name="psA", bufs=3, space="PSUM") as ppA,
                    tc.tile_pool(name="psB", bufs=2, space="PSUM") as ppB,
                ):
                    s = make_stream("l1", l1, xt, ixp, sgp)
                    for g in range((CHUNKS + GRP - 1) // GRP):
                        gs = min(GRP, CHUNKS - g * GRP)
                        psT = ppA.tile([P, gs * P], f32, space="PSUM",
                                       name=f"ps_l1_{g}", tag="psA")
                        nc.tensor.matmul(
                            psT[:], lhsT=iota_t[:, :P],
                            rhs=zeros_t[:, :gs * P],
                            start=True, stop=False, skip_group_check=True)
                        descs = l1["sched"][g]
                        for j, d in enumerate(descs):
                            st = s["ensure"](d["r"], d["call"])
                            w_ = d["nk"] * P
                            sel = sel_build(f"sel_l1_{g}_{j}", s["rel"],
                                            s["w"], d["sel"], w=w_)
                            c0 = (d["k0"] - g * GRP) * P
                            nc.tensor.matmul(
                                psT[:, c0:c0 + w_],
                                lhsT=st[:, d["blk"] * P:
                                        (d["blk"] + 1) * P],
                                rhs=sel[:],
                                start=False,
                                stop=(j == len(descs) - 1),
                                skip_group_check=True)
                        at = wp.tile([P, gs * P], bf16, name=f"at1_{g}",
                                     tag="at")
                        nc.scalar.copy(at[:], psT[:])
                        h_ps = ppB.tile([P, gs * P], f32, space="PSUM",
                                        name=f"h1_{g}", tag="psB")
                        for kk in range(gs):
                            nc.tensor.matmul(
                                h_ps[:, kk * P:(kk + 1) * P],
                                lhsT=at[:, kk * P:(kk + 1) * P],
                                rhs=w1_sb[:], start=True, stop=True)
                        h_sb = wp.tile([P, gs * P], bf16, name=f"h1s_{g}",
                                       tag="h")
                        nc.scalar.activation(
                            h_sb[:], h_ps[:],
                            mybir.ActivationFunctionType.Relu)
                        for kk in range(gs):
                            k = g * GRP + kk
                            nc.sync.dma_start(
                                h_slice[k * P:(k + 1) * P, :],
                                h_sb[:, kk * P:(kk + 1) * P])

            GRP2 = 8

            def run_layer2():
                with (
                    tc.tile_pool(name="ix_l2", bufs=1) as ixp,
                    tc.tile_pool(name="sg_l2", bufs=3) as sgp,
                    tc.tile_pool(name="psL2", bufs=3, space="PSUM") as ppL,
                    tc.tile_pool(name="psU2", bufs=2, space="PSUM") as ppU,
                ):
                    s = make_stream("l2", l2, h_slice[:], ixp, sgp)
                    nsel2 = [0]
                    for g in range(GCHUNKS // GRP2):
                        psT = ppL.tile([P, GRP2 * P], f32, space="PSUM",
                                       name=f"ps_l2_{g}", tag="psL")
                        for hb in range(GRP2 * P // 512):
                            nc.tensor.matmul(
                                psT[:, hb * 512:(hb + 1) * 512],
                                lhsT=iota_t[:, :P],
                                rhs=zeros_t[:, :512],
                                start=True, stop=False,
                                skip_group_check=True)
                        descs = l2["sched"][g]
                        for j, d in enumerate(descs):
                            st = s["ensure"](d["r"], d["call"])
                            nsel2[0] += 1
                            w_ = d["nk"] * P
                            sel = sel_build(f"sel_l2_{g}_{j}", s["rel"],
                                            s["w"], d["sel"], w=w_,
                                            eng=(nc.gpsimd
                                                 if nsel2[0] % 6 == 0
                                                 else None))
                            c0 = (d["k0"] - g * GRP2) * P
                            nc.tensor.matmul(
                                psT[:, c0:c0 + w_],
                                lhsT=st[:, d["blk"] * P:
                                        (d["blk"] + 1) * P],
                                rhs=sel[:],
                                start=False,
                                stop=(j == len(descs) - 1),
                                skip_group_check=True)
                        at = wp.tile([P, GRP2 * P], bf16, name=f"at2_{g}",
                                     tag="at2")
                        nc.scalar.copy(at[:], psT[:])
                        uv_ps = ppU.tile([P, GRP2 * 4], f32, space="PSUM",
                                         name=f"uvp_{g}", tag="psU")
                        for kk in range(GRP2):
                            nc.tensor.matmul(
                                uv_ps[:, kk * 4:(kk + 1) * 4],
                                lhsT=at[:, kk * P:(kk + 1) * P],
                                rhs=wu_sb[:], start=True, stop=True)
                        nc.scalar.copy(
                            uv_full[:, g * GRP2 * 4:(g + 1) * GRP2 * 4],
                            uv_ps[:])
                nc.sync.dma_start(uv_part[:], uv_full[:])

            run_layer1()
            run_layer2()
            # expand uv_red [TABROWS, 4] into zero-padded 256B-row uv_tab;
            # round-robin the big writes over 4 engines' DMA queues
            engs = [nc.sync, nc.scalar, nc.gpsimd, nc.sync]
            with tc.tile_pool(name="expp", bufs=1) as exp_p:
                bigs = []
                for b in range(4):
                    big = exp_p.tile([P, CHUNKS * P], bf16,
                                     name=f"exp_big_{b}")
                    nc.scalar.memzero(big[:])
                    bigs.append(big)
                nc.gpsimd.collective_compute(
                    "ReduceScatter", mybir.AluOpType.add,
                    replica_groups=[list(range(NCORES))],
                    ins=[uv_part[:]], outs=[uv_rs[:]])
                nc.gpsimd.collective_compute(
                    "AllGather", mybir.AluOpType.bypass,
                    replica_groups=[list(range(NCORES))],
                    ins=[uv_rs[:]], outs=[uv_red[:]])
                for i in range(NCORES):
                    big = bigs[i % 4]
                    big3 = big[:].rearrange("p (g c) -> p g c", c=P)
                    nc.sync.dma_start(
                        big3[:, :, 0:4],
                        uv_red[i * SLOTS:(i + 1) * SLOTS, :])
                    engs[i % 4].dma_start(
                        uv_tab[i * SLOTS:(i + 1) * SLOTS, :], big[:])

            # decode: u and v streams into one [2, 128] psum per pair chunk
            with (
                tc.tile_pool(name="psD", bufs=2, space="PSUM") as ppD,
                tc.tile_pool(name="ix_dec", bufs=1) as ixd,
                tc.tile_pool(name="sg_dec", bufs=2) as sgd,
            ):
                su = make_stream("u", du, uv_tab[:], ixd, sgd)
                sv = make_stream("v", dv, uv_tab[:], ixd, sgd)
                for g in range(dchunks // 4):
                    psD = ppD.tile([2, 4 * P], f32, space="PSUM",
                                   name=f"psd_{g}", tag="psD")
                    nc.tensor.matmul(
                        psD[:], lhsT=iota_t[:, :2], rhs=zeros_t[:, :4 * P],
                        start=True, stop=False, skip_group_check=True)
                    descs = ([(su, 0, d) for d in du["sched"][g]]
                             + [(sv, 2, d) for d in dv["sched"][g]])
                    for j, (s, off, d) in enumerate(descs):
                        st = s["ensure"](d["r"], d["call"])
                        w_ = d["nk"] * P
                        sel = sel_build(f"sel_d{off}_{g}_{j}", s["rel"],
                                        s["w"], d["sel"], w=w_,
                                        eng=(nc.gpsimd if j % 4 == 3
                                             else None))
                        cl = d["blk"] * P + off
                        c0 = (d["k0"] - g * 4) * P
                        nc.tensor.matmul(
                            psD[:, c0:c0 + w_],
                            lhsT=st[:, cl:cl + 2], rhs=sel[:],
                            start=False, stop=(j == len(descs) - 1),
                            skip_group_check=True)
                    osb = wp.tile([2, 4 * P], f32, name=f"osb_{g}",
                                  tag="osb")
                    nc.scalar.copy(osb[:], psD[:])
                    nc.sync.dma_start(
                        out_d[:, g * 4 * P:(g + 1) * 4 * P], osb[:])

    nc.compile()

    # ---------- stage inputs & run ----------
    iota_np = np.broadcast_to(np.arange(MAXNK * P, dtype=np.float32)[None, :],
                              (P, MAXNK * P)).astype(bf).copy()
    wcat_np = np.ascontiguousarray(
        np.concatenate([Wlin[:, :P].T, Wlin[:, P:].T], axis=1))
    wu_np = (W2 @ wcat_np).astype(bf)
    w1_np = W1.astype(bf)
    in_maps = []
    for c in range(NCORES):
        m = {"x_tab": x_tab, "W1r": w1_np, "Wu": wu_np, "iota": iota_np}
        for key, pr in (("l1", l1), ("l2", l2), ("u", du), ("v", dv)):
            m[f"rel_{key}"] = np.ascontiguousarray(pr["rel"][c])
            m[f"w_{key}"] = np.ascontiguousarray(pr["wgt"][c])
            for r in range(4):
                if (key, r) in idx_arr:
                    m[f"idx_{key}_{r}"] = idx_arr[(key, r)][c]
        in_maps.append(m)

    if globals().get("RUN_MODE", "hw") == "sim":
        results = _run_sim(nc, in_maps)
    else:
        res = run_bass_kernel_spmd(nc, in_maps, core_ids=list(range(NCORES)),
                                   trace=globals().get("TRACE", False))
        globals()["LAST_EXEC_NS"] = res.exec_time_ns
        results = res.results

    out = np.zeros((npairs, 2), np.float32)
    for c in range(NCORES):
        o2 = results[c]["out_dec"]                       # [2, per_core]
        m = (pair_slot >= c * per_core) & (pair_slot < (c + 1) * per_core)
        sl = pair_slot[m] - c * per_core
        out[m] = o2[:, sl].T
    return out
